# revision 46
# baseline (speedup 1.0000x reference)
"""DetConB loss kernel for Trainium2 (8 NeuronCores, SPMD batch-parallel).

Math (per view v in {0,1}, preds p_v, with T = concat([t1, t2]) all-gathered):
  l[m, u]   = (p̂_v[m] · t̂[u]) / temp                       (4096 x 8192 per view)
  masked    : own-batch intra-view positives get -1e9 before softmax
  LSE[m]    = log sum_u exp(l[m, u])
  ce[m]     = w[m] * (LSE[m] - (1/npos[m]) * sum_s so[m,s] l_diag[m,s])
  loss      = mean_m ce_view0[m] + mean_m ce_view1[m]

Each core handles 32 batches (512 rows) of both views against the full 8192
targets; the scalar partials are summed on host (the "all-reduce").

Since normalized logits are bounded by 1/temp, LSE needs no max pass: the ACT
engine computes exp(s_m*x - 1/temp) with a per-partition scale AND accumulates
the row sums in the same instruction. Matmuls run in float32r (full-rate fp32).
Targets stream in column-chunks; each chunk is squared, column-summed via a
ones-matmul (broadcast across partitions), scaled by exp(-0.5*ln(ss)), and fed
to the PE. Per-core column permutation puts this core's own-batch blocks at a
fixed location so one SPMD program serves all 8 cores.
"""

import sys

for _p in ("/opt/trn_rl_repo", "/root/.axon_site/_ro/trn_rl_repo"):
    if _p not in sys.path:
        sys.path.append(_p)

import numpy as np

import concourse.bacc as bacc
import concourse.mybir as mybir
import concourse.tile as tile
from concourse.bass_utils import run_bass_kernel_spmd

BS, NR, DIM = 256, 16, 256
NCORES = 8
BPC = BS // NCORES            # batches per core = 32
M = BPC * NR                  # local rows = 512
NM = M // 128                 # m-tiles = 4
U = 2 * BS * NR               # total target cols = 8192
KC = DIM // 128               # contraction chunks = 2
P = 128
NEG = -1.0e9

# column chunks
CHS = [2048, 2048, 2048, 2048]
COFF = [0]
for _w in CHS:
    COFF.append(COFF[-1] + _w)
assert COFF[-1] == U
NCH = len(CHS)
CHMAX = max(CHS)
# own-batch diag blocks sit at the END of each 4096-half (host permutation):
# t1-half own block [3584, 4096) -> chunk 1 @ offset 1536
# t2-half own block [7680, 8192) -> chunk 3 @ offset 1536
MASKNEG_AT = {0: 1, 1: 3}     # view -> chunk holding its intra-view diag
NUMER_AT = {0: 3, 1: 1}       # view -> chunk holding its label diag
DIAG_OFF = {1: 1536, 3: 1536}

f32 = mybir.dt.float32
f32r = mybir.dt.float32r
AF = mybir.ActivationFunctionType
OP = mybir.AluOpType
AX = mybir.AxisListType

# packed aux inputs (per-partition column offsets)
# auxe: needed before the first main group (pred norms + temperature)
A_PNAT = (0, 1024)            # pnat v0/v1 (NM*DIM each)
A_TEMP = 2048
AUXEW = 2064
# auxm: index/mask data, needed from chunk-0 v1 onward
A_MI = (0, 512)               # maskidx v0/v1 (NM*P each)
A_LI = (1024, 1536)           # labidx v0/v1
A_PR = (2048, 2112)           # prep v0/v1 (NM*NR each)
A_PIND = (2176, 2180)         # pind v0/v1 (NM each)
AUXMW = 2192

LAST_EXEC_TIME_NS = None
_COMPILED = {}


def _patch_act_tables():
    """Force Exp and Ln to resolve to the combined natural_log_exp set so the
    Exp<->Ln alternation doesn't thrash ACT table loads. Only the cached
    func->set MAPPING is edited; set indices (and the real table data walrus
    loads) stay untouched."""
    from concourse.hw_specs import get_activation_tables
    tabs = get_activation_tables("gen3")
    for name, funcs in tabs.items():
        if name != "natural_log_exp_and_others":
            funcs.discard(AF.Exp)
            funcs.discard(AF.Ln)


def _build_nc():
    _patch_act_tables()
    nc = bacc.Bacc()
    tT_d = nc.dram_tensor("tT", [P, KC, U], f32, kind="ExternalInput")
    pT_d = [nc.dram_tensor(f"pT{v}", [P, KC * M], f32r, kind="ExternalInput") for v in range(2)]
    auxe_d = nc.dram_tensor("auxe", [P, AUXEW], f32, kind="ExternalInput")
    auxm_d = nc.dram_tensor("auxm", [P, AUXMW], f32, kind="ExternalInput")
    out_d = nc.dram_tensor("out", [1, 1], f32, kind="ExternalOutput")

    with tile.TileContext(nc) as tc:
        with (
            tc.tile_pool(name="const", bufs=1) as cp,
            tc.tile_pool(name="work", bufs=1) as wp,
            tc.tile_pool(name="psum", bufs=2, space="PSUM") as pp,
        ):
            ones_f32 = cp.tile([P, P], f32, tag="ones_f32")
            nc.vector.memset(ones_f32[:], 1.0)
            ones = cp.tile([P, P], f32r, tag="ones")
            nc.vector.tensor_copy(ones[:], ones_f32[:])

            traws = {}

            def segs(c):
                # chunk 0 is cut in halves so its norm pipeline fills faster
                w = CHS[c]
                return [(0, w // 2), (w // 2, w - w // 2)] if c == 0 else [(0, w)]

            def dma_block(c):
                w = CHS[c]
                traw = wp.tile([P, KC, w], f32, tag="traw", bufs=3)
                for (o, sw) in segs(c):
                    for k in range(KC):
                        nc.sync.dma_start(traw[:, k, o:o + sw],
                                          tT_d[:, k, COFF[c] + o:COFF[c] + o + sw])
                traws[c] = traw

            tnorms = {}

            def norm_block(c):
                """Square, column-sum (ones matmul, result broadcast across
                partitions by using a full ones stationary), rsqrt via
                exp(-.5 ln), scale: t̂ chunk ready for the PE."""
                w = CHS[c]
                traw = traws.pop(c)
                sq = wp.tile([P, KC, w], f32r, tag="sq")
                bc = pp.tile([P, w], f32, tag="grp")
                lnbc = wp.tile([P, w], f32, tag="lnbc")
                scl = wp.tile([P, w], f32, tag="scl")
                tnorm = wp.tile([P, KC, w], f32r, tag="tnorm", bufs=2)
                for (o, sw) in segs(c):
                    ssl = slice(o, o + sw)
                    nc.vector.tensor_tensor(sq[:, 0, ssl], traw[:, 0, ssl], traw[:, 0, ssl], OP.mult)
                    nc.gpsimd.tensor_tensor(sq[:, 1, ssl], traw[:, 1, ssl], traw[:, 1, ssl], OP.mult)
                    for j in range(sw // 512):
                        js = slice(o + j * 512, o + (j + 1) * 512)
                        for k in range(KC):
                            nc.tensor.matmul(bc[:, js], ones[:], sq[:, k, js],
                                             start=(k == 0), stop=(k == KC - 1))
                    nc.scalar.activation(lnbc[:, ssl], bc[:, ssl], AF.Ln, bias=0.0)
                    nc.scalar.activation(scl[:, ssl], lnbc[:, ssl], AF.Exp, bias=0.0, scale=-0.5)
                    nc.vector.tensor_tensor(tnorm[:, 0, ssl], traw[:, 0, ssl], scl[:, ssl], OP.mult)
                    nc.gpsimd.tensor_tensor(tnorm[:, 1, ssl], traw[:, 1, ssl], scl[:, ssl], OP.mult)
                tnorms[c] = tnorm

            dma_block(0)
            auxe = cp.tile([P, AUXEW], f32, tag="auxe")
            nc.sync.dma_start(auxe[:], auxe_d[:])
            pT = []
            for v in range(2):
                t = cp.tile([P, KC * M], f32r, tag=f"pT{v}")
                nc.sync.dma_start(t[:], pT_d[v][:])
                pT.append(t)
            dma_block(1)
            auxm = cp.tile([P, AUXMW], f32, tag="auxm")
            nc.sync.dma_start(auxm[:], auxm_d[:])
            norm_block(0)

            recip_t = cp.tile([P, 1], f32, tag="recip_t")
            nc.vector.reciprocal(recip_t[:], auxe[:, A_TEMP:A_TEMP + 1])
            neg_rt = cp.tile([P, 1], f32, tag="neg_rt")
            nc.vector.tensor_scalar_mul(neg_rt[:], recip_t[:], -1.0)

            # s_all[:, v*4+mt] = 1 / (temp * |p_v[m]|)  per partition row
            # (fused square+row-sum per m-tile keeps the chain short: the
            # first main exp instruction is gated on s_all)
            s_all = cp.tile([P, 2 * NM], f32, tag="s_all")
            ssq = cp.tile([P, 2 * NM], f32, tag="ssq")
            for v in range(2):
                for mt in range(NM):
                    pm = auxe[:, A_PNAT[v] + mt * DIM:A_PNAT[v] + (mt + 1) * DIM]
                    junk = wp.tile([P, DIM], f32, tag="sttjunk", bufs=2)
                    nc.vector.scalar_tensor_tensor(
                        junk[:], pm, 1.0, pm, OP.mult, OP.mult,
                        accum_out=ssq[:, v * NM + mt: v * NM + mt + 1],
                    )
            nc.vector.tensor_scalar_max(ssq[:], ssq[:], 1e-24)
            lnss = wp.tile([P, 2 * NM], f32, tag="lnss")
            nc.scalar.activation(lnss[:], ssq[:], AF.Ln, bias=0.0)
            nc.scalar.activation(s_all[:], lnss[:], AF.Exp, bias=0.0, scale=-0.5)
            nc.vector.tensor_scalar(s_all[:], s_all[:], recip_t[:], None, OP.mult)

            # index-derived masks / weights (needed from chunk 2 on; emitted
            # between chunk-0 main groups)
            maskneg = []
            labmask = []
            npos = cp.tile([P, 2 * NM], f32, tag="npos")
            obj_area = cp.tile([P, 2 * NM], f32, tag="obj_area")

            def mask_block(v):
                mi = auxm[:, A_MI[v]:A_MI[v] + NM * P]
                li = auxm[:, A_LI[v]:A_LI[v] + NM * P]
                pr = auxm[:, A_PR[v]:A_PR[v] + NM * NR]
                mn = cp.tile([P, NM * P], f32, tag=f"mn{v}")
                lm = cp.tile([P, NM * P], f32, tag=f"lm{v}")
                for mt in range(NM):
                    sl = slice(mt * P, (mt + 1) * P)
                    pcol = auxm[:, A_PIND[v] + mt:A_PIND[v] + mt + 1]
                    nc.vector.tensor_scalar(mn[:, sl], mi[:, sl], pcol, NEG, OP.is_equal, OP.mult)
                    nc.vector.tensor_scalar(
                        lm[:, sl], li[:, sl], pcol, None, OP.is_equal, OP.add,
                        accum_out=npos[:, v * NM + mt: v * NM + mt + 1],
                    )
                    tmp16 = wp.tile([P, NR], f32, tag="tmp16")
                    nc.vector.tensor_scalar(
                        tmp16[:], pr[:, mt * NR:(mt + 1) * NR], pcol, None, OP.is_equal, OP.add,
                        accum_out=obj_area[:, v * NM + mt: v * NM + mt + 1],
                    )
                maskneg.append(mn)
                labmask.append(lm)

            def weights_block():
                npos_c = cp.tile([P, 2 * NM], f32, tag="npos_c")
                nc.vector.tensor_scalar_max(npos_c[:], npos[:], 1.0)
                recip_np = cp.tile([P, 2 * NM], f32, tag="recip_np")
                nc.vector.reciprocal(recip_np[:], npos_c[:])
                gate = cp.tile([P, 2 * NM], f32, tag="gate")
                nc.vector.tensor_scalar_min(gate[:], npos[:], 1.0)
                recip_oa = cp.tile([P, 2 * NM], f32, tag="recip_oa")
                nc.vector.reciprocal(recip_oa[:], obj_area[:])
                w = cp.tile([P, 2 * NM], f32, tag="w")
                nc.vector.tensor_tensor(w[:], gate[:], recip_oa[:], OP.mult)
                return recip_np, w

            numer = cp.tile([P, 2 * NM], f32, tag="numer")
            zpart = cp.tile([P, 2 * NM * NCH], f32, tag="zpart")

            # ---------- chunk loop ----------
            recip_np = w_tile = None
            for c in range(NCH):
                tnorm = tnorms.pop(c)
                wc = CHS[c]
                nj = wc // 512
                for v in range(2):
                    if c == 0 and v == 1:
                        mask_block(0)
                        mask_block(1)
                        recip_np, w_tile = weights_block()
                    if v == 0 and c + 2 < NCH:
                        dma_block(c + 2)
                    if v == 1 and c + 1 < NCH:
                        norm_block(c + 1)
                    for mt in range(NM):
                        grp = pp.tile([P, wc], f32, tag="grp")
                        for k in range(KC):
                            lhs = pT[v][:, k * M + mt * P: k * M + (mt + 1) * P]
                            for j in range(nj):
                                js = slice(j * 512, (j + 1) * 512)
                                nc.tensor.matmul(grp[:, js], lhs, tnorm[:, k, js],
                                                 start=(k == 0), stop=(k == KC - 1))
                        msl = slice(mt * P, (mt + 1) * P)
                        if MASKNEG_AT[v] == c:
                            gsl = slice(DIAG_OFF[c] + mt * P, DIAG_OFF[c] + (mt + 1) * P)
                            nc.vector.tensor_tensor(grp[:, gsl], grp[:, gsl], maskneg[v][:, msl], OP.add)
                        if NUMER_AT[v] == c:
                            gsl = slice(DIAG_OFF[c] + mt * P, DIAG_OFF[c] + (mt + 1) * P)
                            prod = wp.tile([P, P], f32, tag="prod", bufs=2)
                            nc.vector.tensor_tensor(prod[:], labmask[v][:, msl], grp[:, gsl], OP.mult)
                            nc.vector.reduce_sum(
                                numer[:, v * NM + mt: v * NM + mt + 1], prod[:], axis=AX.X
                            )
                        ev = wp.tile([P, wc], f32, tag="ev")
                        zi = (v * NM + mt) * NCH + c
                        nc.scalar.activation(
                            ev[:], grp[:], AF.Exp,
                            bias=neg_rt[:], scale=s_all[:, v * NM + mt: v * NM + mt + 1],
                            accum_out=zpart[:, zi:zi + 1],
                        )

            # ---------- final reduction ----------
            z = wp.tile([P, 2 * NM], f32, tag="z")
            nc.vector.reduce_sum(z[:], zpart[:].rearrange("p (j c) -> p j c", c=NCH), axis=AX.X)
            lse0 = wp.tile([P, 2 * NM], f32, tag="lse0")
            nc.scalar.activation(lse0[:], z[:], AF.Ln, bias=0.0)
            # LSE = lse0 + 1/temp ; ce = w * (LSE - numer * s * recip_np)
            nc.vector.tensor_scalar(lse0[:], lse0[:], recip_t[:], None, OP.add)
            t1 = wp.tile([P, 2 * NM], f32, tag="t1")
            nc.vector.tensor_tensor(t1[:], numer[:], s_all[:], OP.mult)
            nc.vector.tensor_tensor(t1[:], t1[:], recip_np[:], OP.mult)
            ce = wp.tile([P, 2 * NM], f32, tag="ce")
            nc.vector.tensor_tensor(ce[:], lse0[:], t1[:], OP.subtract)
            nc.vector.tensor_tensor(ce[:], ce[:], w_tile[:], OP.mult)
            ce_rows = wp.tile([P, 1], f32, tag="ce_rows")
            nc.vector.reduce_sum(ce_rows[:], ce[:], axis=AX.X)
            nc.vector.tensor_scalar_mul(ce_rows[:], ce_rows[:], 1.0 / (BS * NR))
            fin = pp.tile([P, CHMAX], f32, tag="grp")
            nc.tensor.matmul(fin[0:1, 0:1], ce_rows[:], ones_f32[:, 0:1], start=True, stop=True)
            res = wp.tile([1, 1], f32, tag="res")
            nc.scalar.copy(res[:], fin[0:1, 0:1])
            nc.sync.dma_start(out_d[:], res[:])

    nc.compile()
    return nc


def _prep_core_inputs(c, pred1, pred2, target1, target2, pind1, pind2, tind1, tind2, temperature):
    b0 = c * BPC
    preds = (pred1, pred2)
    pinds = (pind1, pind2)
    # view 0 intra-mask from tind1, labels from tind2; view 1 swapped
    mask_src = (tind1, tind2)
    lab_src = (tind2, tind1)

    m = {}
    auxe = np.zeros((P, AUXEW), np.float32)
    auxm = np.zeros((P, AUXMW), np.float32)
    # targets: [t1 | t2] halves, each permuted so this core's 512 columns come LAST
    own = np.arange(b0 * NR, (b0 + BPC) * NR)
    rest = np.concatenate([np.arange(0, b0 * NR), np.arange((b0 + BPC) * NR, BS * NR)])
    perm = np.concatenate([rest, own])
    t1f = target1.reshape(BS * NR, DIM)[perm]
    t2f = target2.reshape(BS * NR, DIM)[perm]
    T = np.concatenate([t1f, t2f], axis=0)                     # [U, DIM]
    m["tT"] = np.ascontiguousarray(
        T.T.reshape(KC, P, U).transpose(1, 0, 2)
    ).astype(np.float32)                                       # [P, KC, U]

    for v in range(2):
        x = preds[v][b0:b0 + BPC].reshape(M, DIM)
        auxe[:, A_PNAT[v]:A_PNAT[v] + NM * DIM] = (
            x.reshape(NM, P, DIM).transpose(1, 0, 2).reshape(P, NM * DIM)
        )
        m[f"pT{v}"] = np.ascontiguousarray(
            x.T.reshape(KC, P, M).transpose(1, 0, 2).reshape(P, KC * M)
        ).astype(np.float32)

        pi = pinds[v][b0:b0 + BPC].astype(np.float32)          # [BPC, NR]
        auxm[:, A_PIND[v]:A_PIND[v] + NM] = pi.reshape(M).reshape(NM, P).T
        auxm[:, A_PR[v]:A_PR[v] + NM * NR] = (
            np.repeat(pi[:, None, :], NR, axis=1).reshape(M, NR).reshape(NM, P, NR)
            .transpose(1, 0, 2).reshape(P, NM * NR)
        )

        for aoff, idx_src in ((A_MI[v], mask_src[v]), (A_LI[v], lab_src[v])):
            E = np.full((M, P), -1.0, np.float32)
            ti = idx_src[b0:b0 + BPC].astype(np.float32)
            for beta in range(BPC):
                rows = slice(beta * NR, (beta + 1) * NR)
                col = (beta % 8) * NR
                E[rows, col:col + NR] = ti[beta]
            auxm[:, aoff:aoff + NM * P] = (
                E.reshape(NM, P, P).transpose(1, 0, 2).reshape(P, NM * P)
            )

    auxe[:, A_TEMP] = np.asarray(temperature).reshape(-1)[0]
    m["auxe"] = auxe
    m["auxm"] = auxm
    return m


def kernel(pred1, pred2, target1, target2, pind1, pind2, tind1, tind2, temperature):
    global LAST_EXEC_TIME_NS
    import os
    trace = bool(int(os.environ.get("KERNEL_TRACE", "0")))
    if "nc" not in _COMPILED:
        _COMPILED["nc"] = _build_nc()
    nc = _COMPILED["nc"]

    args = (np.asarray(pred1), np.asarray(pred2), np.asarray(target1), np.asarray(target2),
            np.asarray(pind1), np.asarray(pind2), np.asarray(tind1), np.asarray(tind2),
            np.asarray(temperature))
    in_maps = [_prep_core_inputs(c, *args) for c in range(NCORES)]
    res = run_bass_kernel_spmd(nc, in_maps, core_ids=list(range(NCORES)), trace=trace)
    LAST_EXEC_TIME_NS = res.exec_time_ns
    total = sum(float(res.results[c]["out"][0, 0]) for c in range(NCORES))
    return np.float32(total)
